# revision 46
# baseline (speedup 1.0000x reference)
"""HSTU positional encoder on Trainium2, SPMD across 8 NeuronCores.

out[t] = seq_embeddings[t] * sqrt(D) + pos_weight[pos[t]]

pos[t] derives from the ragged structure (seq_offsets / seq_lengths):
within a sequence of length L it runs L, L-1, ..., 1 -- contiguous.

Layout ("chunked cache"):
- Host groups tokens into 128-row-aligned "pieces": a piece holds one
  sequence's tokens whose pos values span one aligned 128-block
  (partition p <-> pos 128*j + p).  C consecutive pieces of one
  sequence form a "group" whose table rows are ONE contiguous aligned
  C*128-row window of the table, so a group needs a single dynamic
  offset -- no gather, one tensor op per group.
- Groups are w-sorted and split evenly over the 8 cores; each core
  caches its small table window in SBUF once.  The group structure is
  padded to be identical on every core (SPMD shares one program).
- Everything on the wire is int8 in "e-units": host computes
  s = max|emb*sqrt(D)|/127, sends emb8 = round(emb*sqrt(D)/s) and the
  table pre-divided by s (fp16).  The device adds the cached table
  window and emits int8 (hardware converts round-to-nearest-even);
  host multiplies by s and scatters back to token order.  Error is
  <= s/2 ~ 0.5 on values up to ~127 -> rel err ~4e-3, well inside the
  2e-2 gate and identical to the fp16-output variant of this kernel.
- Per-core HBM traffic ~9.5MB (~26us of DMA) vs 48MB fp32 naive.
- Engine schedule: "YC" groups are loaded by a casting SWDGE DMA
  (int8 wire -> fp16 SBUF, the upconvert rides free on DMA slack),
  added in fp16 on the DVE (2x mode), and downconverted fp16->int8 on
  the scalar engine (or the DVE tensor_scalar path for a few groups);
  "X" groups use a plain int8 load and a direct int8+fp16->int8 DVE
  add (1x mode).  The mix is chosen so DVE, scalar, and DMA all stay
  near the same busy time.  All group offsets are loaded into DVE
  registers by ONE batched values_load (the per-piece loads + drains
  cost ~25us of sequencer time in the previous revision), and all
  tiles are kept live so only true data deps remain.
"""

import numpy as np

import concourse.bacc as bacc
import concourse.bass as bass
import concourse.mybir as mybir
import concourse.tile as tile
from concourse.bass_utils import run_bass_kernel_spmd

N_CORES = 8
TOTAL = 65536
D = 512
TABLE_ROWS = 8192
PART = 128
ALPHA = float(np.sqrt(D))
FP = mybir.dt.float16

# tunables
C = 4             # pieces (128-pos-blocks) per group
N_X = 9           # groups on the direct int8 path (plain load + 1x add)
N_TSOUT = 0       # y groups whose downconvert runs on DVE tensor_scalar
ST_RING = "gp"    # store dispatch ring
KSPLIT = 4        # groups below this index read the small early cache
LD_RING = "sync"  # load dispatch ring
HALVE = 2         # first groups processed in two halves (earlier start)
HALVE_TAIL = 1    # also halve the last group (earlier final store)

_cache: dict = {}


def _xy_sets(ngc, n_x):
    """x/y layout: y groups at early odd positions, pure-x tail."""
    ny = ngc - n_x
    ys = set(range(1, 2 * ny, 2)) if 2 * ny <= ngc else \
        set(range(ngc - ny, ngc))
    if 2 * ny > ngc:
        ys = set(np.round(np.linspace(1, ngc - 4, ny)).astype(int))
    xs = set(range(ngc)) - ys
    return xs, ys


def _halved(ngc):
    """Groups processed in two halves: the first HALVE (earlier compute
    start) and the last (earlier final store)."""
    h = list(range(min(HALVE, ngc)))
    if HALVE_TAIL and ngc - 1 not in h:
        h.append(ngc - 1)
    return h


def _build_nc(ngc, nb, n_x, n_tsout, ksplit, sza, szb):
    """ngc groups/core, each C pieces (FD = C*D).  The table cache is two
    tiles: groups [0, ksplit) read cache_a (sza blocks, loaded first so
    early adds don't wait for the whole cache), groups [ksplit, ngc)
    read cache_b (szb blocks)."""
    kd = C * D
    halved = _halved(ngc)
    hpos = {g: i for i, g in enumerate(halved)}
    xs, ys = _xy_sets(ngc, min(n_x, ngc))
    ts_out = set()
    for g in sorted(ys, reverse=True):
        if len(ts_out) >= n_tsout:
            break
        ts_out.add(g)

    nc = bacc.Bacc("TRN2", target_bir_lowering=False, debug=False)
    embp = nc.dram_tensor("embp", [ngc * C * PART, D], mybir.dt.int8,
                          kind="ExternalInput")
    tca = nc.dram_tensor("tca", [PART, sza * D], FP, kind="ExternalInput")
    tcb = nc.dram_tensor("tcb", [PART, szb * D], FP, kind="ExternalInput")
    boff = nc.dram_tensor("boff", [1, ngc + len(halved)], mybir.dt.int32,
                          kind="ExternalInput")
    outp = nc.dram_tensor("outp", [ngc * C * PART, D], mybir.dt.int8,
                          kind="ExternalOutput")

    emb_v = embp.ap().rearrange("(g p c) d -> g p c d", p=PART, c=C)
    out_v = outp.ap().rearrange("(g p c) d -> g p c d", p=PART, c=C)
    h = kd // 2

    with tile.TileContext(nc) as tc:
        with (
            tc.tile_pool(name="fixed", bufs=1) as fixp,
            tc.tile_pool(name="sbuf", bufs=ngc) as pool,
        ):
            boff_sb = fixp.tile([1, ngc + len(halved)], mybir.dt.int32,
                                tag="boff")
            nc.sync.dma_start(boff_sb[:], boff.ap())
            cache_a = fixp.tile([PART, sza * D], FP, tag="ca")
            nc.scalar.dma_start(cache_a[:], tca.ap())
            cache_b = fixp.tile([PART, szb * D], FP, tag="cb")
            nc.gpsimd.dma_start(cache_b[:], tcb.ap())

            def cache_of(g):
                return cache_a if g < ksplit else cache_b

            def halves(g):
                return 2 if g in hpos else 1

            # All loads up front: the first two ride the (otherwise idle)
            # sync HWDGE ring so they land with minimal contention; the
            # rest go through gpsimd SWDGE, whose ~1us/dispatch desc-gen
            # naturally paces them so they never starve the small
            # critical-chain DMAs (boff/cache_a).  cache_b is slotted a
            # few dispatches in -- early enough for group KSPLIT, late
            # enough not to crowd the ramp.  The first HALVE groups load
            # in two half-DMAs so compute can start on the first half.
            e8s = []
            for g in range(ngc):
                t = pool.tile([PART, kd], mybir.dt.int8,
                              tag="e8" if g in xs else "e8y")
                ld_eng = nc.sync
                if halves(g) == 2:
                    for a in range(2):
                        ld_eng.dma_start(
                            t[:, a * h:(a + 1) * h].rearrange(
                                "p (c d) -> p c d", c=C // 2),
                            emb_v[g][:, a * (C // 2):(a + 1) * (C // 2)])
                else:
                    ld_eng.dma_start(
                        t[:].rearrange("p (c d) -> p c d", c=C), emb_v[g])
                e8s.append(t)

            # scalar upconverts for every y group, ahead of any
            # downconvert so a C-in never stalls behind a C-out.
            tiles = []
            for g in range(ngc):
                if g in xs:
                    tiles.append(e8s[g])
                    continue
                e16 = pool.tile([PART, kd], FP, tag="e16")
                for a in range(halves(g)):
                    w = kd // halves(g)
                    nc.scalar.copy(
                        e16[:, a * w:(a + 1) * w],
                        e8s[g][:, a * w:(a + 1) * w])
                tiles.append(e16)

            # Two batched register loads, one per cache tile; each covers
            # that tile's full-window offsets plus the half-window extras
            # of its halved groups (packed contiguously by the host).  The
            # B load is emitted after group 0's adds so its sequencer time
            # hides behind compute.  max_val is the full-window bound --
            # half-window offsets may exceed it at runtime, but reads stay
            # in-tile by construction and runtime checks are off.
            ha = [g for g in halved if g < ksplit]
            hb = [g for g in halved if g >= ksplit]
            na, nbg = ksplit + len(ha), (ngc - ksplit) + len(hb)

            def sv_of(g, half):
                if g < ksplit:
                    return va[g] if not half else va[ksplit + ha.index(g)]
                return (vb[g - ksplit] if not half
                        else vb[(ngc - ksplit) + hb.index(g)])

            _, va = nc.values_load_multi_w_load_instructions(
                boff_sb[0:1, 0:na],
                engines=[mybir.EngineType.DVE],
                min_val=0, max_val=(sza - C) * D,
                skip_runtime_bounds_check=True)
            vb = None

            st_eng = {"gp": nc.gpsimd, "scalar": nc.scalar,
                      "sync": nc.sync}[ST_RING]
            for g in range(ngc):
                if g == 1:
                    # deferred so its sequencer time hides behind compute
                    _, vb = nc.values_load_multi_w_load_instructions(
                        boff_sb[0:1, na:na + nbg],
                        engines=[mybir.EngineType.DVE],
                        min_val=0, max_val=(szb - C) * D,
                        skip_runtime_bounds_check=True)
                t = tiles[g]
                csb = cache_of(g)
                if halves(g) == 2:
                    split_store = g in xs
                    for a in range(2):
                        sv = sv_of(g, half=(a == 1))
                        nc.vector.tensor_add(
                            t[:, a * h:(a + 1) * h], t[:, a * h:(a + 1) * h],
                            csb[:, bass.ds(sv, h)])
                        if split_store:
                            st_eng.dma_start(
                                out_v[g][:, a * (C // 2):(a + 1) * (C // 2)],
                                t[:, a * h:(a + 1) * h].rearrange(
                                    "p (c d) -> p c d", c=C // 2))
                    if split_store:
                        continue
                else:
                    nc.vector.tensor_add(
                        t[:], t[:], csb[:, bass.ds(sv_of(g, False), kd)])
                if g in xs:
                    st_t = t
                else:
                    o8 = pool.tile([PART, kd], mybir.dt.int8, tag="o8")
                    if g in ts_out:
                        nc.vector.tensor_scalar_mul(o8[:], t[:], 1.0)
                    else:
                        nc.scalar.copy(o8[:], t[:])
                    st_t = o8
                st_eng.dma_start(
                    out_v[g], st_t[:].rearrange("p (c d) -> p c d", c=C))
    nc.compile()
    return nc


def _get_nc(ngc, nb, ksplit, sza, szb):
    key = (ngc, nb, C, N_X, N_TSOUT, ST_RING, LD_RING, HALVE,
           HALVE_TAIL, ksplit, sza, szb)
    if key not in _cache:
        _cache[key] = _build_nc(ngc, nb, N_X, N_TSOUT, ksplit, sza, szb)
    return _cache[key]


def _plan(seq_lengths, seq_offsets):
    """Group plan.  A group = C consecutive 128-blocks of one sequence's
    pos range [128*w0, 128*(w0+C)); per-block jobs give the token range
    mapped to each partition.  Returns (core_groups, ngc, nb) or None."""
    lens = np.asarray(seq_lengths).astype(np.int64)
    offs = np.asarray(seq_offsets).astype(np.int64)
    groups = []
    for s in range(len(lens)):
        L = int(lens[s])
        hi = min(L, TABLE_ROWS - 1)
        if L > hi:
            return None
        start = int(offs[s])
        lo = hi - L + 1
        w_lo, w_hi = lo // PART, hi // PART
        nw = w_hi - w_lo + 1
        npad = ((nw + C - 1) // C) * C
        for w0 in range(w_lo, w_lo + npad, C):
            jobs = []
            for j in range(w0, w0 + C):
                wlo = max(PART * j, lo)
                whi = min(PART * j + PART - 1, hi)
                if whi < wlo or j > w_hi:
                    jobs.append(None)
                    continue
                jobs.append((start + (hi - whi), whi - PART * j,
                             whi - wlo + 1))
            groups.append((w0, jobs))
    groups.sort(key=lambda x: x[0])
    ngc = (len(groups) + N_CORES - 1) // N_CORES
    per_core = [list(a) for a in
                np.array_split(np.arange(len(groups)), N_CORES)]
    core_groups = [[groups[i] for i in idxs] for idxs in per_core]
    nb = C
    for cg in core_groups:
        ws = [w for (w, _) in cg]
        nb = max(nb, max(ws) - min(ws) + C)
    # static split of each core's (w-sorted) groups over the two cache
    # tiles: groups [0, k) read tile A, the rest read tile B
    k = min(KSPLIT, ngc - 1)
    sza, sb = C, nb
    for cg in core_groups:
        blo = min(w for (w, _) in cg)
        ws = [w - blo for (w, _) in cg]
        sza = max(sza, max(ws[:k]) + C)
        if len(ws) > k:
            sb = min(sb, min(ws[k:]))
    szb = nb - sb
    return core_groups, ngc, nb, k, sza, sb, szb


def _blockify(table_s, b0, nblk):
    """Table rows [128*b0, 128*(b0+nblk)) in [PART, nblk*D] layout."""
    rows = table_s[b0 * PART:(b0 + nblk) * PART]
    if rows.shape[0] < nblk * PART:
        rows = np.pad(rows, ((0, nblk * PART - rows.shape[0]), (0, 0)))
    return np.ascontiguousarray(
        rows.reshape(nblk, PART, D).transpose(1, 0, 2).reshape(
            PART, nblk * D))


def _core_inputs(cg, ngc, nb, ksplit, sza, sb, szb, emb8, table_s):
    nt = ngc * C
    halved = _halved(ngc)
    ha = [g for g in halved if g < ksplit]
    hb = [g for g in halved if g >= ksplit]
    blo = min(w for (w, _) in cg) if cg else 0
    gidx = np.zeros((nt, PART), np.int64)
    valid = np.zeros((nt, PART), bool)
    # boff layout: [A fulls, A half-extras, B fulls, B half-extras]
    boff_arr = np.zeros((1, ngc + len(halved)), np.int32)

    def col(g, half=False):
        if g < ksplit:
            return (ksplit + ha.index(g)) if half else g
        base = ksplit + len(ha)
        return (base + (ngc - ksplit) + hb.index(g)) if half \
            else base + (g - ksplit)

    full_off = np.zeros(ngc, np.int32)
    for gi, (w0, jobs) in enumerate(cg):
        rel = w0 - blo - (0 if gi < ksplit else sb)
        cap = (sza if gi < ksplit else szb) - C
        full_off[gi] = min(max(rel, 0), cap) * D
        boff_arr[0, col(gi)] = full_off[gi]
        for r, job in enumerate(jobs):
            if job is None:
                continue
            tok0, p_hi, n = job
            t = gi * C + r
            ps = np.arange(p_hi, p_hi - n, -1)
            gidx[t, ps] = tok0 + np.arange(n)
            valid[t, ps] = True
    for g in halved:
        boff_arr[0, col(g, half=True)] = full_off[g] + (C // 2) * D
    gidx_f = gidx.reshape(ngc, C, PART).transpose(0, 2, 1).reshape(-1)
    valid_f = valid.reshape(ngc, C, PART).transpose(0, 2, 1).reshape(-1)
    embp = np.ascontiguousarray(emb8[gidx_f])
    tca = _blockify(table_s, blo, sza)
    tcb = _blockify(table_s, blo + sb, szb)
    return ({"embp": embp, "tca": tca, "tcb": tcb, "boff": boff_arr},
            gidx_f, valid_f)


def _run(max_seq_len, seq_lengths, seq_offsets, seq_embeddings, pos_weight,
         trace=False):
    embf = np.asarray(seq_embeddings, dtype=np.float32) * ALPHA
    total = embf.shape[0]
    plan = _plan(seq_lengths, seq_offsets)
    if plan is None:
        # degenerate shapes (sequence longer than the table): host fallback
        lens = np.asarray(seq_lengths).astype(np.int64)
        offs = np.asarray(seq_offsets).astype(np.int64)
        tok = np.arange(total, dtype=np.int64)
        seg = np.searchsorted(offs, tok, side="right") - 1
        high = np.minimum(lens, TABLE_ROWS - 1)
        pos = np.clip(high[seg] - (tok - offs[seg]), 0, TABLE_ROWS - 1)
        full = embf + np.asarray(pos_weight, np.float32)[pos]
        return full, None
    s = max(float(np.abs(embf).max()) / 127.0, 1e-12)
    emb8 = np.clip(np.rint(embf / s), -127, 127).astype(np.int8)
    table_s = (np.asarray(pos_weight, np.float32) / s).astype(np.float16)
    core_groups, ngc, nb, ksplit, sza, sb, szb = plan
    built = [_core_inputs(cg, ngc, nb, ksplit, sza, sb, szb, emb8, table_s)
             for cg in core_groups]
    in_maps = [b[0] for b in built]
    res = run_bass_kernel_spmd(_get_nc(ngc, nb, ksplit, sza, szb), in_maps,
                               list(range(N_CORES)), trace=trace)
    full = np.empty((total, D), np.float32)
    for c in range(N_CORES):
        _, gidx_f, valid_f = built[c]
        outp = np.asarray(res.results[c]["outp"])
        full[gidx_f[valid_f]] = outp[valid_f]
    full *= s
    return full, res


def kernel(max_seq_len, seq_lengths, seq_offsets, seq_embeddings, pos_weight):
    full, _ = _run(max_seq_len, seq_lengths, seq_offsets, seq_embeddings,
                   pos_weight)
    return full


# revision 47
# speedup vs baseline: 1.0161x; 1.0161x over previous
"""HSTU positional encoder on Trainium2, SPMD across 8 NeuronCores.

out[t] = seq_embeddings[t] * sqrt(D) + pos_weight[pos[t]]

pos[t] derives from the ragged structure (seq_offsets / seq_lengths):
within a sequence of length L it runs L, L-1, ..., 1 -- contiguous.

Layout ("chunked cache"):
- Host groups tokens into 128-row-aligned "pieces": a piece holds one
  sequence's tokens whose pos values span one aligned 128-block
  (partition p <-> pos 128*j + p).  C consecutive pieces of one
  sequence form a "group" whose table rows are ONE contiguous aligned
  C*128-row window of the table, so a group needs a single dynamic
  offset -- no gather, one tensor op per group.
- Groups are w-sorted and split evenly over the 8 cores; each core
  caches its small table window in SBUF once.  The group structure is
  padded to be identical on every core (SPMD shares one program).
- Everything on the wire is int8 in "e-units": host computes
  s = max|emb*sqrt(D)|/127, sends emb8 = round(emb*sqrt(D)/s) and the
  table pre-divided by s (fp16).  The device adds the cached table
  window and emits int8 (hardware converts round-to-nearest-even);
  host multiplies by s and scatters back to token order.  Error is
  <= s/2 ~ 0.5 on values up to ~127 -> rel err ~4e-3, well inside the
  2e-2 gate and identical to the fp16-output variant of this kernel.
- Per-core HBM traffic ~9.5MB (~26us of DMA) vs 48MB fp32 naive.
- Engine schedule: "YC" groups are loaded by a casting SWDGE DMA
  (int8 wire -> fp16 SBUF, the upconvert rides free on DMA slack),
  added in fp16 on the DVE (2x mode), and downconverted fp16->int8 on
  the scalar engine (or the DVE tensor_scalar path for a few groups);
  "X" groups use a plain int8 load and a direct int8+fp16->int8 DVE
  add (1x mode).  The mix is chosen so DVE, scalar, and DMA all stay
  near the same busy time.  All group offsets are loaded into DVE
  registers by ONE batched values_load (the per-piece loads + drains
  cost ~25us of sequencer time in the previous revision), and all
  tiles are kept live so only true data deps remain.
"""

import numpy as np

import concourse.bacc as bacc
import concourse.bass as bass
import concourse.mybir as mybir
import concourse.tile as tile
from concourse.bass_utils import run_bass_kernel_spmd

N_CORES = 8
TOTAL = 65536
D = 512
TABLE_ROWS = 8192
PART = 128
ALPHA = float(np.sqrt(D))
FP = mybir.dt.float16

# tunables
C = 4             # pieces (128-pos-blocks) per group
N_X = 10          # groups on the direct int8 path (plain load + 1x add)
N_TSOUT = 0       # y groups whose downconvert runs on DVE tensor_scalar
ST_RING = "gp"    # store dispatch ring
KSPLIT = 4        # groups below this index read the small early cache
LD_RING = "sync"  # load dispatch ring
HALVE = 2         # first groups processed in two halves (earlier start)
HALVE_TAIL = 1    # also halve the last group (earlier final store)

_cache: dict = {}


def _xy_sets(ngc, n_x):
    """x/y layout: y groups at early odd positions, pure-x tail."""
    ny = ngc - n_x
    ys = set(range(1, 2 * ny, 2)) if 2 * ny <= ngc else \
        set(range(ngc - ny, ngc))
    if 2 * ny > ngc:
        ys = set(np.round(np.linspace(1, ngc - 4, ny)).astype(int))
    xs = set(range(ngc)) - ys
    return xs, ys


def _halved(ngc):
    """Groups processed in two halves: the first HALVE (earlier compute
    start) and the last (earlier final store)."""
    h = list(range(min(HALVE, ngc)))
    if HALVE_TAIL and ngc - 1 not in h:
        h.append(ngc - 1)
    return h


def _build_nc(ngc, nb, n_x, n_tsout, ksplit, sza, szb):
    """ngc groups/core, each C pieces (FD = C*D).  The table cache is two
    tiles: groups [0, ksplit) read cache_a (sza blocks, loaded first so
    early adds don't wait for the whole cache), groups [ksplit, ngc)
    read cache_b (szb blocks)."""
    kd = C * D
    halved = _halved(ngc)
    hpos = {g: i for i, g in enumerate(halved)}
    xs, ys = _xy_sets(ngc, min(n_x, ngc))
    ts_out = set()
    for g in sorted(ys, reverse=True):
        if len(ts_out) >= n_tsout:
            break
        ts_out.add(g)

    nc = bacc.Bacc("TRN2", target_bir_lowering=False, debug=False)
    embp = nc.dram_tensor("embp", [ngc * C * PART, D], mybir.dt.int8,
                          kind="ExternalInput")
    tca = nc.dram_tensor("tca", [PART, sza * D], FP, kind="ExternalInput")
    tcb = nc.dram_tensor("tcb", [PART, szb * D], FP, kind="ExternalInput")
    boff = nc.dram_tensor("boff", [1, ngc + len(halved)], mybir.dt.int32,
                          kind="ExternalInput")
    outp = nc.dram_tensor("outp", [ngc * C * PART, D], mybir.dt.int8,
                          kind="ExternalOutput")

    emb_v = embp.ap().rearrange("(g p c) d -> g p c d", p=PART, c=C)
    out_v = outp.ap().rearrange("(g p c) d -> g p c d", p=PART, c=C)
    h = kd // 2

    with tile.TileContext(nc) as tc:
        with (
            tc.tile_pool(name="fixed", bufs=1) as fixp,
            tc.tile_pool(name="sbuf", bufs=ngc) as pool,
        ):
            boff_sb = fixp.tile([1, ngc + len(halved)], mybir.dt.int32,
                                tag="boff")
            nc.sync.dma_start(boff_sb[:], boff.ap())
            cache_a = fixp.tile([PART, sza * D], FP, tag="ca")
            nc.scalar.dma_start(cache_a[:], tca.ap())
            cache_b = fixp.tile([PART, szb * D], FP, tag="cb")
            nc.gpsimd.dma_start(cache_b[:], tcb.ap())

            def cache_of(g):
                return cache_a if g < ksplit else cache_b

            def halves(g):
                return 2 if g in hpos else 1

            # All loads up front: the first two ride the (otherwise idle)
            # sync HWDGE ring so they land with minimal contention; the
            # rest go through gpsimd SWDGE, whose ~1us/dispatch desc-gen
            # naturally paces them so they never starve the small
            # critical-chain DMAs (boff/cache_a).  cache_b is slotted a
            # few dispatches in -- early enough for group KSPLIT, late
            # enough not to crowd the ramp.  The first HALVE groups load
            # in two half-DMAs so compute can start on the first half.
            e8s = []
            for g in range(ngc):
                t = pool.tile([PART, kd], mybir.dt.int8,
                              tag="e8" if g in xs else "e8y")
                ld_eng = nc.sync
                if halves(g) == 2:
                    for a in range(2):
                        ld_eng.dma_start(
                            t[:, a * h:(a + 1) * h].rearrange(
                                "p (c d) -> p c d", c=C // 2),
                            emb_v[g][:, a * (C // 2):(a + 1) * (C // 2)])
                else:
                    ld_eng.dma_start(
                        t[:].rearrange("p (c d) -> p c d", c=C), emb_v[g])
                e8s.append(t)

            # scalar upconverts for every y group, ahead of any
            # downconvert so a C-in never stalls behind a C-out.
            tiles = []
            for g in range(ngc):
                if g in xs:
                    tiles.append(e8s[g])
                    continue
                e16 = pool.tile([PART, kd], FP, tag="e16")
                for a in range(halves(g)):
                    w = kd // halves(g)
                    nc.scalar.activation(
                        e16[:, a * w:(a + 1) * w],
                        e8s[g][:, a * w:(a + 1) * w],
                        mybir.ActivationFunctionType.Identity)
                tiles.append(e16)

            # Two batched register loads, one per cache tile; each covers
            # that tile's full-window offsets plus the half-window extras
            # of its halved groups (packed contiguously by the host).  The
            # B load is emitted after group 0's adds so its sequencer time
            # hides behind compute.  max_val is the full-window bound --
            # half-window offsets may exceed it at runtime, but reads stay
            # in-tile by construction and runtime checks are off.
            ha = [g for g in halved if g < ksplit]
            hb = [g for g in halved if g >= ksplit]
            na, nbg = ksplit + len(ha), (ngc - ksplit) + len(hb)

            def sv_of(g, half):
                if g < ksplit:
                    return va[g] if not half else va[ksplit + ha.index(g)]
                return (vb[g - ksplit] if not half
                        else vb[(ngc - ksplit) + hb.index(g)])

            _, va = nc.values_load_multi_w_load_instructions(
                boff_sb[0:1, 0:na],
                engines=[mybir.EngineType.DVE],
                min_val=0, max_val=(sza - C) * D,
                skip_runtime_bounds_check=True)
            _, vb = nc.values_load_multi_w_load_instructions(
                boff_sb[0:1, na:na + nbg],
                engines=[mybir.EngineType.DVE],
                min_val=0, max_val=(szb - C) * D,
                skip_runtime_bounds_check=True)

            st_eng = {"gp": nc.gpsimd, "scalar": nc.scalar,
                      "sync": nc.sync}[ST_RING]
            for g in range(ngc):
                t = tiles[g]
                csb = cache_of(g)
                if halves(g) == 2:
                    split_store = g in xs
                    for a in range(2):
                        sv = sv_of(g, half=(a == 1))
                        nc.vector.tensor_add(
                            t[:, a * h:(a + 1) * h], t[:, a * h:(a + 1) * h],
                            csb[:, bass.ds(sv, h)])
                        if split_store:
                            st_eng.dma_start(
                                out_v[g][:, a * (C // 2):(a + 1) * (C // 2)],
                                t[:, a * h:(a + 1) * h].rearrange(
                                    "p (c d) -> p c d", c=C // 2))
                    if split_store:
                        continue
                else:
                    nc.vector.tensor_add(
                        t[:], t[:], csb[:, bass.ds(sv_of(g, False), kd)])
                if g in xs:
                    st_t = t
                else:
                    o8 = pool.tile([PART, kd], mybir.dt.int8, tag="o8")
                    if g in ts_out:
                        nc.vector.tensor_scalar_mul(o8[:], t[:], 1.0)
                    else:
                        nc.scalar.activation(
                            o8[:], t[:],
                            mybir.ActivationFunctionType.Identity)
                    st_t = o8
                st_eng.dma_start(
                    out_v[g], st_t[:].rearrange("p (c d) -> p c d", c=C))
    nc.compile()
    return nc


def _get_nc(ngc, nb, ksplit, sza, szb):
    key = (ngc, nb, C, N_X, N_TSOUT, ST_RING, LD_RING, HALVE,
           HALVE_TAIL, ksplit, sza, szb)
    if key not in _cache:
        _cache[key] = _build_nc(ngc, nb, N_X, N_TSOUT, ksplit, sza, szb)
    return _cache[key]


def _plan(seq_lengths, seq_offsets):
    """Group plan.  A group = C consecutive 128-blocks of one sequence's
    pos range [128*w0, 128*(w0+C)); per-block jobs give the token range
    mapped to each partition.  Returns (core_groups, ngc, nb) or None."""
    lens = np.asarray(seq_lengths).astype(np.int64)
    offs = np.asarray(seq_offsets).astype(np.int64)
    groups = []
    for s in range(len(lens)):
        L = int(lens[s])
        hi = min(L, TABLE_ROWS - 1)
        if L > hi:
            return None
        start = int(offs[s])
        lo = hi - L + 1
        w_lo, w_hi = lo // PART, hi // PART
        nw = w_hi - w_lo + 1
        npad = ((nw + C - 1) // C) * C
        for w0 in range(w_lo, w_lo + npad, C):
            jobs = []
            for j in range(w0, w0 + C):
                wlo = max(PART * j, lo)
                whi = min(PART * j + PART - 1, hi)
                if whi < wlo or j > w_hi:
                    jobs.append(None)
                    continue
                jobs.append((start + (hi - whi), whi - PART * j,
                             whi - wlo + 1))
            groups.append((w0, jobs))
    groups.sort(key=lambda x: x[0])
    ngc = (len(groups) + N_CORES - 1) // N_CORES
    per_core = [list(a) for a in
                np.array_split(np.arange(len(groups)), N_CORES)]
    core_groups = [[groups[i] for i in idxs] for idxs in per_core]
    nb = C
    for cg in core_groups:
        ws = [w for (w, _) in cg]
        nb = max(nb, max(ws) - min(ws) + C)
    # static split of each core's (w-sorted) groups over the two cache
    # tiles: groups [0, k) read tile A, the rest read tile B
    k = min(KSPLIT, ngc - 1)
    sza, sb = C, nb
    for cg in core_groups:
        blo = min(w for (w, _) in cg)
        ws = [w - blo for (w, _) in cg]
        sza = max(sza, max(ws[:k]) + C)
        if len(ws) > k:
            sb = min(sb, min(ws[k:]))
    szb = nb - sb
    return core_groups, ngc, nb, k, sza, sb, szb


def _blockify(table_s, b0, nblk):
    """Table rows [128*b0, 128*(b0+nblk)) in [PART, nblk*D] layout."""
    rows = table_s[b0 * PART:(b0 + nblk) * PART]
    if rows.shape[0] < nblk * PART:
        rows = np.pad(rows, ((0, nblk * PART - rows.shape[0]), (0, 0)))
    return np.ascontiguousarray(
        rows.reshape(nblk, PART, D).transpose(1, 0, 2).reshape(
            PART, nblk * D))


def _core_inputs(cg, ngc, nb, ksplit, sza, sb, szb, emb8, table_s):
    nt = ngc * C
    halved = _halved(ngc)
    ha = [g for g in halved if g < ksplit]
    hb = [g for g in halved if g >= ksplit]
    blo = min(w for (w, _) in cg) if cg else 0
    gidx = np.zeros((nt, PART), np.int64)
    valid = np.zeros((nt, PART), bool)
    # boff layout: [A fulls, A half-extras, B fulls, B half-extras]
    boff_arr = np.zeros((1, ngc + len(halved)), np.int32)

    def col(g, half=False):
        if g < ksplit:
            return (ksplit + ha.index(g)) if half else g
        base = ksplit + len(ha)
        return (base + (ngc - ksplit) + hb.index(g)) if half \
            else base + (g - ksplit)

    full_off = np.zeros(ngc, np.int32)
    for gi, (w0, jobs) in enumerate(cg):
        rel = w0 - blo - (0 if gi < ksplit else sb)
        cap = (sza if gi < ksplit else szb) - C
        full_off[gi] = min(max(rel, 0), cap) * D
        boff_arr[0, col(gi)] = full_off[gi]
        for r, job in enumerate(jobs):
            if job is None:
                continue
            tok0, p_hi, n = job
            t = gi * C + r
            ps = np.arange(p_hi, p_hi - n, -1)
            gidx[t, ps] = tok0 + np.arange(n)
            valid[t, ps] = True
    for g in halved:
        boff_arr[0, col(g, half=True)] = full_off[g] + (C // 2) * D
    gidx_f = gidx.reshape(ngc, C, PART).transpose(0, 2, 1).reshape(-1)
    valid_f = valid.reshape(ngc, C, PART).transpose(0, 2, 1).reshape(-1)
    embp = np.ascontiguousarray(emb8[gidx_f])
    tca = _blockify(table_s, blo, sza)
    tcb = _blockify(table_s, blo + sb, szb)
    return ({"embp": embp, "tca": tca, "tcb": tcb, "boff": boff_arr},
            gidx_f, valid_f)


def _run(max_seq_len, seq_lengths, seq_offsets, seq_embeddings, pos_weight,
         trace=False):
    embf = np.asarray(seq_embeddings, dtype=np.float32) * ALPHA
    total = embf.shape[0]
    plan = _plan(seq_lengths, seq_offsets)
    if plan is None:
        # degenerate shapes (sequence longer than the table): host fallback
        lens = np.asarray(seq_lengths).astype(np.int64)
        offs = np.asarray(seq_offsets).astype(np.int64)
        tok = np.arange(total, dtype=np.int64)
        seg = np.searchsorted(offs, tok, side="right") - 1
        high = np.minimum(lens, TABLE_ROWS - 1)
        pos = np.clip(high[seg] - (tok - offs[seg]), 0, TABLE_ROWS - 1)
        full = embf + np.asarray(pos_weight, np.float32)[pos]
        return full, None
    s = max(float(np.abs(embf).max()) / 127.0, 1e-12)
    emb8 = np.clip(np.rint(embf / s), -127, 127).astype(np.int8)
    table_s = (np.asarray(pos_weight, np.float32) / s).astype(np.float16)
    core_groups, ngc, nb, ksplit, sza, sb, szb = plan
    built = [_core_inputs(cg, ngc, nb, ksplit, sza, sb, szb, emb8, table_s)
             for cg in core_groups]
    in_maps = [b[0] for b in built]
    res = run_bass_kernel_spmd(_get_nc(ngc, nb, ksplit, sza, szb), in_maps,
                               list(range(N_CORES)), trace=trace)
    full = np.empty((total, D), np.float32)
    for c in range(N_CORES):
        _, gidx_f, valid_f = built[c]
        outp = np.asarray(res.results[c]["outp"])
        full[gidx_f[valid_f]] = outp[valid_f]
    full *= s
    return full, res


def kernel(max_seq_len, seq_lengths, seq_offsets, seq_embeddings, pos_weight):
    full, _ = _run(max_seq_len, seq_lengths, seq_offsets, seq_embeddings,
                   pos_weight)
    return full
